# revision 27
# baseline (speedup 1.0000x reference)
"""BetaTCVAE loss kernel for 8 Trainium2 NeuronCores.

Math: reference computes
    kl_loss = sum(kl)
    log_qz_prob[i,j,l] = -0.5*((z_i_l - m_j_l)^2 * exp(-v_j_l) + v_j_l + LOG2PI)
    log_qz_product[i]  = sum_l logsumexp_j log_qz_prob[i,j,l]
    log_qz[i]          = logsumexp_j sum_l log_qz_prob[i,j,l]
    out = (BETA-1)*mean_i(log_qz - log_qz_product) + kl_loss

Key transform: with w = exp(-v),
    log_qz_prob[i,j,l] = a[j,l]*z2[i,l] + b[j,l]*z[i,l] + g[j,l]
      a = -w/2, b = w*m, g = -(w*m^2 + v + LOG2PI)/2, z2 = z^2

Coefficients are pre-scaled on host so matmul PSUM holds
y = ENC_A*arg + ENC_B (ENC_A = 1024/ln2, ENC_B = 15360): round(y) IS the
fp16 bit pattern of exp(arg) (Schraudolph).

Phase B issues BLOCK-DIAGONAL stationaries: lhsT[(l,k), (ls,is)] is
z_k[i,l] on the block diagonal, so one K=96 matmul computes args for
32 latents x 4 batch rows at once. K=96 keeps the PE array's activity
high enough for the hardware clock ramp (K=3 matmuls never leave the
~1.2GHz p-state; K=96 reach full speed), and phase B then shares the
phase-A coefficient tensors as moving data -- no per-l DMA stream.
Off-block entries get tiny +/-1e-30 noise instead of zeros to keep
switching activity up; the products (~1e-26) are harmless.

The O(B^2*L) exp work is split across engines per tile:
  * ScalarE tiles: native Exp (scale/bias decode of y) with fused
    accum_out reduction over j.
  * VectorE tiles: one tensor_scalar (add SIG, max 0) converting fp32
    PSUM -> int16 SBUF = fp16 exp bits (HW convert is round-to-nearest,
    SIG tunes away the Schraudolph bias); optionally GPSIMD halves the
    bitcast-fp16 tile (tensor_tensor add) before a VectorE tensor_reduce
    finishes the j sum.

Everything after ln(G) is a full sum, so per-partition partials
(sum_l ln G, lq per half, and h=sum kl) are DMA'd out and summed on
host along with the closed-form encoding-offset correction.
"""

import os
import sys
from contextlib import ExitStack

import numpy as np

for _p in ("/opt/trn_rl_repo", "/root/.axon_site/_ro/trn_rl_repo"):
    if os.path.isdir(_p) and _p not in sys.path:
        sys.path.append(_p)

import concourse.bass as bass
import concourse.tile as tile
from concourse import mybir

BETA = 6.0
LOG_2PI = float(np.log(2.0 * np.pi))
F32 = mybir.dt.float32
BF16 = mybir.dt.bfloat16
F16 = mybir.dt.float16
I16 = mybir.dt.int16
AF = mybir.ActivationFunctionType
ALU = mybir.AluOpType

ENC_A = 1024.0 / float(np.log(2.0))     # y = ENC_A*arg + ENC_B
ENC_B = 15360.0                          # = 15 * 1024 (fp16 exponent bias)
ENC_C = float(np.log(2.0)) / 1024.0     # decode scale: arg = (y-ENC_B)*ENC_C
SIG = -58.9135                           # Schraudolph bias correction
PHASEA_AT = 2                            # run phase A after this many B tiles
LG = 16                                  # latents per block-diag stationary
IG = 8                                   # batch rows per block-diag stationary


def build_nc(B=2048, L=64, BC=256, split_waits=True):
    PI = 128
    assert LG * IG == PI and 3 * LG <= PI
    JT = min(512, B)
    njc = B // JT
    KC = 3 * LG                          # stationary contraction dim (96)
    nkc = (3 * L) // KC                  # coefficient groups (2)
    nlg = L // LG                        # latent groups (2)
    nig = BC // IG                       # i groups per latent group (64)
    ntiles = nlg * nig                   # phase-B tiles (128)
    nit = BC // PI                       # phase-A row tiles (2)

    nc = bass.Bass()
    wd_d = nc.declare_dram_parameter("wd", [nlg, KC, nig * PI], BF16, False)
    zs_d = nc.declare_dram_parameter("zs", [nkc, KC, BC], BF16, False)
    coefs_d = nc.declare_dram_parameter("coefs", [nkc, KC, B], BF16, False)
    out_d = nc.declare_dram_parameter("out", [PI, 1 + nit], F32, True)

    with tile.TileContext(nc) as tc, ExitStack() as ctx:
        const_pool = ctx.enter_context(tc.tile_pool(name="const", bufs=1))
        es_pool = ctx.enter_context(tc.tile_pool(name="es", bufs=2))
        i16_pool = ctx.enter_context(tc.tile_pool(name="i16", bufs=2))
        h_pool = ctx.enter_context(tc.tile_pool(name="h", bufs=4))
        small = ctx.enter_context(tc.tile_pool(name="small", bufs=1))
        # Separate PSUM pools for the two consumers: Tile serializes
        # cross-engine readers of one pool buffer, so ScalarE's and
        # VectorE's shares must be distinct tiles to drain in parallel.
        CA = 1536                        # ScalarE's share of each tile's j
        ND = B - CA                      # VectorE's share
        psA = ctx.enter_context(tc.tile_pool(name="psA", bufs=2, space="PSUM"))
        psD = ctx.enter_context(tc.tile_pool(name="psD", bufs=2, space="PSUM"))

        # --- persistent loads ---
        # coefs/wd live twice: rows [0:KC) for PE band 0 and rows
        # [64:64+KC) for band 1 (walrus requires stationary+moving base
        # partition == tile_position row).
        zs_t, coefs_t, wd_t = [], [], []
        for k in range(nkc):
            t2 = const_pool.tile([64 + KC, B], BF16, tag=f"cs{k}", name=f"cs{k}")
            nc.sync.dma_start(out=t2[0:KC, :], in_=coefs_d[k])
            nc.sync.dma_start(out=t2[64:64 + KC, :], in_=coefs_d[k])
            coefs_t.append(t2)
            t = const_pool.tile([KC, BC], BF16, tag=f"zs{k}", name=f"zs{k}")
            nc.sync.dma_start(out=t[:], in_=zs_d[k])
            zs_t.append(t)
        for lg in range(nlg):
            t = const_pool.tile([64 + KC, nig * PI], BF16, tag=f"wd{lg}",
                                name=f"wd{lg}")
            nc.sync.dma_start(out=t[0:KC, :], in_=wd_d[lg])
            nc.sync.dma_start(out=t[64:64 + KC, :], in_=wd_d[lg])
            wd_t.append(t)

        g_all = small.tile([PI, ntiles], F32, tag="gall", name="gall")
        g_act = small.tile([PI, ntiles], F32, tag="gact", name="gact")
        lq_t = {}
        biasb = small.tile([PI, 1], F32, tag="biasb")
        nc.gpsimd.memset(biasb[:], -ENC_B * ENC_C)

        def phase_a(it):
            # log_qz: S = sum_l y_l = ENC_A * (sum_l arg_l) + L*ENC_B
            spa = psA.tile([PI, CA], F32, tag="rA", name=f"spa{it}")
            spd = psD.tile([PI, ND], F32, tag="rD", name=f"spd{it}")
            for k in range(nkc):
                lhsT = zs_t[k][:, it * PI:(it + 1) * PI]
                nc.tensor.matmul(
                    spd[:], lhsT, coefs_t[k][0:KC, CA:B],
                    start=(k == 0), stop=(k == nkc - 1))
                for jc in range(CA // JT):
                    nc.tensor.matmul(
                        spa[:, jc * JT:(jc + 1) * JT],
                        lhsT,
                        coefs_t[k][0:KC, jc * JT:(jc + 1) * JT],
                        start=(k == 0),
                        stop=(k == nkc - 1),
                    )
            mx = small.tile([PI, 1], F32, tag=f"mx{it}", name=f"mx{it}")
            mxd = small.tile([PI, 1], F32, tag=f"mxd{it}", name=f"mxd{it}")
            nc.vector.tensor_reduce(mx[:], spa[:], axis=mybir.AxisListType.X,
                                    op=ALU.max)
            nc.vector.tensor_reduce(mxd[:], spd[:], axis=mybir.AxisListType.X,
                                    op=ALU.max)
            nc.vector.tensor_tensor(mx[:], mx[:], mxd[:], ALU.max)
            negmxc = small.tile([PI, 1], F32, tag=f"negmxc{it}",
                                name=f"negmxc{it}")
            nc.scalar.mul(negmxc[:], mx[:], -ENC_C)
            es = es_pool.tile([PI, CA], F32, tag="es", name=f"esA{it}")
            esd = es_pool.tile([PI, ND], F32, tag="esd", name=f"esD{it}")
            sume = small.tile([PI, 1], F32, tag=f"sume{it}", name=f"sume{it}")
            sumd = small.tile([PI, 1], F32, tag=f"sumd{it}", name=f"sumd{it}")
            nc.scalar.activation(es[:], spa[:], AF.Exp, bias=negmxc[:],
                                 scale=ENC_C, accum_out=sume[:])
            nc.scalar.activation(esd[:], spd[:], AF.Exp, bias=negmxc[:],
                                 scale=ENC_C, accum_out=sumd[:])
            nc.vector.tensor_add(sume[:], sume[:], sumd[:])
            lq = small.tile([PI, 1], F32, tag=f"lq{it}", name=f"lq{it}")
            nc.scalar.activation(lq[:], sume[:], AF.Ln)
            mxc = small.tile([PI, 1], F32, tag=f"mxc{it}", name=f"mxc{it}")
            nc.scalar.mul(mxc[:], mx[:], ENC_C)
            nc.vector.tensor_add(lq[:], lq[:], mxc[:])
            lq_t[it] = lq
            # lq is short by L*ENC_B*ENC_C vs ln(sum_j exp(S)); host corrects.

        # --- phase B: G[(ls,is), tile] = sum_j exp(arg) ---
        # Split-drain: every PSUM tile is consumed by ScalarE (cols
        # [0:ca), native exp + accum -> g_act) and VectorE (cols [ca:B),
        # Schraudolph convert) IN PARALLEL on disjoint banks, so the
        # drain beats the PE fill and the PE stays continuously busy --
        # which is what lets the clock ramp to the high p-state.
        # ca alternates 1536/1024 to balance ACT vs DVE load.
        # Matmuls alternate PE row bands (K=48 at rows 0/64) so each
        # tile's LDWEIGHTS overlaps the other band's streaming.
        # The j-sum of the DVE part is ONE fused tensor_tensor_reduce:
        # pairwise f16 add of the two convert halves with accumulator
        # initialized from ScalarE's partial -- emitted one tile late so
        # the convert (which releases PSUM) always leads the DVE queue.
        pend = []
        pend2 = []

        def emit_stt(item):
            e16, nd, to, gcol = item
            nc.vector.scalar_tensor_tensor(
                out=to[:, :nd // 2],
                in0=e16[:, :nd // 2].bitcast(F16),
                scalar=0.0,
                in1=e16[:, nd // 2:nd].bitcast(F16),
                op0=ALU.add, op1=ALU.add,
                accum_out=gcol)

        def emit_stt2(item):
            hed, to2, gacol = item
            nc.vector.scalar_tensor_tensor(
                out=to2[:],
                in0=hed[:, :CA // 4],
                scalar=0.0,
                in1=hed[:, CA // 4:CA // 2],
                op0=ALU.add, op1=ALU.add,
                accum_out=gacol)

        k_flat = 0
        for lg in range(nlg):
            for ig in range(nig):
                if k_flat == PHASEA_AT:
                    for it2 in range(nit):
                        phase_a(it2)
                apA = psA.tile([PI, CA], F32, tag="rA")
                apD = psD.tile([PI, ND], F32, tag="rD")
                band = 64 * (k_flat % 2)
                lhsT = wd_t[lg][band:band + KC, ig * PI:(ig + 1) * PI]
                # D-chunk first: the convert (which gates the psD buffer
                # handoff) gets its input as early as possible.
                nc.tensor.matmul(
                    apD[:], lhsT, coefs_t[lg][band:band + KC, CA:B],
                    start=True, stop=True, tile_position=(band, 0))
                for jc in range(CA // JT):
                    nc.tensor.matmul(
                        apA[:, jc * JT:(jc + 1) * JT],
                        lhsT,
                        coefs_t[lg][band:band + KC, jc * JT:(jc + 1) * JT],
                        start=True,
                        stop=True,
                        tile_position=(band, 0),
                    )
                ga = g_act[:, k_flat:k_flat + 1]
                ed = es_pool.tile([PI, CA], BF16, tag="ed")
                nc.scalar.activation(ed[:], apA[:], AF.Exp,
                                     bias=biasb[:], scale=ENC_C)
                # GPSIMD (otherwise idle) halves the ScalarE exp output;
                # a lagged DVE STT finishes that j-sum into g_act. This
                # drops the 283ns accumulator read from every ScalarE
                # activation -- the loop-critical engine.
                hed = h_pool.tile([PI, CA // 2], BF16, tag="hed")
                nc.gpsimd.tensor_tensor(hed[:], ed[:, :CA // 2],
                                        ed[:, CA // 2:], ALU.add)
                e16 = i16_pool.tile([PI, ND], I16, tag="e16")
                nc.vector.tensor_scalar(e16[:], apD[:], SIG, 0.0,
                                        ALU.add, ALU.max)
                to = h_pool.tile([PI, ND // 2], F16, tag="h")
                to2 = h_pool.tile([PI, CA // 4], BF16, tag="to2")
                pend.append((e16, ND, to, g_all[:, k_flat:k_flat + 1]))
                pend2.append((hed, to2, ga))
                if len(pend) >= 2:
                    emit_stt(pend.pop(0))
                if len(pend2) >= 3:
                    emit_stt2(pend2.pop(0))
                k_flat += 1
        while pend:
            emit_stt(pend.pop(0))
        while pend2:
            emit_stt2(pend2.pop(0))
        nc.vector.tensor_add(g_all[:], g_all[:], g_act[:])

        # --- combine: ln(G), free-reduce; DMA per-partition partials ---
        logg = small.tile([PI, ntiles], F32, tag="logg")
        nc.scalar.activation(logg[:], g_all[:], AF.Ln)
        res = small.tile([PI, 1 + nit], F32, tag="res")
        nc.vector.tensor_reduce(res[:, 0:1], logg[:],
                                axis=mybir.AxisListType.X, op=ALU.add)
        for it in range(nit):
            nc.vector.tensor_copy(res[:, 1 + it:2 + it], lq_t[it][:])
        nc.sync.dma_start(out=out_d[:], in_=res[:])

    return _split_multi_waits(nc) if split_waits else nc


def _split_multi_waits(nc):
    """Walrus (gen3 codegen) accepts at most ONE sync-wait per instruction.
    Tile's wait assignment can attach several. Split the extras onto NoOp
    instructions on the same engine immediately before the instruction —
    same-engine streams execute in order, so semantics are preserved."""
    wid = [0]

    def fix_block(b):
        new = []
        for inst in b.instructions:
            si = inst.sync_info
            if si is not None and si.on_wait and len(si.on_wait) > 1:
                for w in si.on_wait[:-1]:
                    wid[0] += 1
                    nop = mybir.InstNoOp(
                        name=f"WSPLIT-{wid[0]}",
                        engine=inst.engine,
                        sync_info=mybir.SyncInfo(on_wait=[w], on_update=[]),
                    )
                    nop.bass_nofuse = True
                    new.append(nop)
                si.on_wait = [si.on_wait[-1]]
            new.append(inst)
        b.instructions[:] = new

    for fn in nc.m.functions:
        for b in fn.blocks:
            fix_block(b)
    return nc


def make_inputs(kl, z_mean, z_logvar, z_sampled, n_cores):
    """Host-side O(B*L) prep: y-encoded coefficients + block-diag z."""
    B, L = kl.shape
    BC = B // n_cores
    PI = 128
    KC = 3 * LG
    nkc = (3 * L) // KC
    nlg = L // LG
    nig = BC // IG

    m = np.asarray(z_mean, dtype=np.float32)
    v = np.asarray(z_logvar, dtype=np.float32)
    z = np.asarray(z_sampled, dtype=np.float32)

    w = np.exp(-v)
    a = ENC_A * (-0.5 * w)
    b = ENC_A * (w * m)
    g = ENC_A * (-0.5 * (w * m * m + v + LOG_2PI)) + ENC_B
    import ml_dtypes
    bf = ml_dtypes.bfloat16
    coefs = np.ascontiguousarray(
        np.stack([a, b, g], 0).transpose(2, 0, 1).reshape(3 * L, B)
        .reshape(nkc, KC, B)).astype(bf)  # [nkc, KC, B], row = l*3+k

    rng = np.random.default_rng(12345)

    in_maps = []
    for c in range(n_cores):
        zc = z[c * BC:(c + 1) * BC]                      # [BC, L]
        arr = np.stack([zc * zc, zc, np.ones_like(zc)], 0)  # [3, BC, L]
        zs = np.ascontiguousarray(
            arr.transpose(2, 0, 1).reshape(3 * L, BC)
            .reshape(nkc, KC, BC)).astype(bf)
        # block-diagonal stationaries: wd[lg, ls*3+k, ig*PI + ls*IG+is]
        # = arr[k, ig*IG+is, lg*LG+ls]; off-block tiny noise keeps the
        # PE power/activity governor at the high clock p-state.
        wd = (rng.integers(0, 2, size=(nlg, KC, nig * PI)) * 2e-30 - 1e-30
              ).astype(np.float32)
        ls_arr = np.arange(LG)
        for lg in range(nlg):
            blk = arr[:, :, lg * LG:(lg + 1) * LG]       # [3, BC, LG]
            # rows ls*3+k ; cols ig*PI + ls*IG + is
            for k in range(3):
                rows = ls_arr * 3 + k                     # [LG]
                colbase = np.arange(nig)[:, None] * PI + ls_arr[None, :] * IG
                for is_ in range(IG):
                    cols = colbase + is_                  # [nig, LG]
                    ivals = blk[k, np.arange(nig)[:, None] * IG + is_, ls_arr[None, :]]
                    wd[lg, rows[None, :].repeat(nig, 0), cols] = ivals
        in_maps.append({
            "wd": np.ascontiguousarray(wd).astype(bf),
            "zs": zs,
            "coefs": coefs,
        })
    return in_maps


_NC_CACHE = {}


def _get_nc(B, L, BC):
    key = (B, L, BC)
    if key not in _NC_CACHE:
        _NC_CACHE[key] = build_nc(B, L, BC)
    return _NC_CACHE[key]


def _enable_jax_cache():
    try:
        import jax
        jax.config.update("jax_compilation_cache_dir", "/tmp/jaxcache")
        jax.config.update("jax_persistent_cache_min_entry_size_bytes", 0)
        jax.config.update("jax_persistent_cache_min_compile_time_secs", 0)
    except Exception:
        pass


def host_total(results, kl, B, L):
    """Combine per-core per-partition partials on host."""
    scale_r = (BETA - 1.0) / float(B)
    tot = 0.0
    for r in results:
        o = np.asarray(r["out"], dtype=np.float64)
        sum_lng = o[:, 0].sum()          # sum_{i,l in core} ln G
        sum_lq = o[:, 1:].sum()          # sum_i lq (encoded)
        tot += scale_r * (sum_lq - sum_lng)
    tot -= (BETA - 1.0) * (L * ENC_B * ENC_C)   # lq encoding offset
    tot += float(np.asarray(kl, dtype=np.float64).sum())
    return np.float32(tot)


def kernel(kl, z_mean, z_logvar, z_sampled):
    from concourse.bass_utils import run_bass_kernel_spmd

    _enable_jax_cache()

    B, L = kl.shape
    n_cores = 8
    BC = B // n_cores
    nc = _get_nc(B, L, BC)
    in_maps = make_inputs(kl, z_mean, z_logvar, z_sampled, n_cores)
    res = run_bass_kernel_spmd(nc, in_maps, list(range(n_cores)))
    return host_total(res.results, kl, B, L)
